# revision 13
# baseline (speedup 1.0000x reference)
"""Trainium2 Bass kernel for MinimalLBS (B=32, T=128, N=2048, J=52, Jb=21, L=16).

Data-parallel over B across 8 NeuronCores (4 samples per core).

Device math per sample (per 128-vertex chunk, t free):
  MAIN (exact, bf16 on PE): fold v_template+homogeneous into the stationary:
     M[n,(i,t)] = sum_{(j,k)} wvh[(j,k),n] * arm[(j,k),(i,t)]
     where wvh[(j,k),n] = wt[k,n]*vth[n,j]  (host-prepped, K=212),
     arm[(j,k),(i,t)] = A[t,k,i,j] (+ translation row at (3,J)).
  CORRECTION (fp8 DoubleRow): dv[n,j,t] (pose+shape offsets, K=206 dual-tile
     over 103 partitions) and ts8[n,(j,i,t)] (K=52 dual-tile) on PE; pm8 =
     ts8*dv on DVE (fp8 out, x32 scale in D8); j-reduction via fp8 identity
     matmuls (ident/32) accumulating straight into the M PSUM bank.

Steady state is DVE-bound: one 1325ns pm8 tensor_tensor per chunk,
back-to-back.  To keep that train unstalled, dv/dvs for chunk g+1 are
computed one window EARLY (software pipeline depth 2 on the correction
inputs), so pm8(g) only ever waits on ts8(g), which lands ~400ns into the
window.  Rodrigues pose features + betas are folded on the HOST into zc8;
PE is pre-warmed with dummy matmuls; gating DMAs are split so chunk 0's
inputs (zc8[0], first 6 n-chunks of d8[0], wt8[0], ar8[0]) land first.
"""

import sys

sys.path.insert(0, "/opt/trn_rl_repo")

import ml_dtypes
import numpy as np

import concourse.bacc as bacc
import concourse.mybir as mybir
import concourse.tile as tile
from concourse import bass_utils, masks

F32 = mybir.dt.float32
BF16 = mybir.dt.bfloat16
FP8 = mybir.dt.float8e4
NPBF16 = ml_dtypes.bfloat16
NPF8 = ml_dtypes.float8_e4m3

B, T, N, JB, J, L = 32, 128, 2048, 21, 52, 16
NCORES = 8
NB = B // NCORES          # samples per core
PF = JB * 9               # 189 pose-feature dims
Z = PF + L                # 205 combined correction coeffs
ZP = 103                  # dual-pair partitions for the z contraction (206>=205)
KA = (J + 1) * 4          # 212 (j,k) rows for the main matmul
NCH = N // 128            # n-chunks per sample
DS = 32.0                 # fp8 scale baked into D8, undone by ident/DS
D8SPLIT = 2 * 128         # first d8[0] DMA piece covers 2 n-chunks

_CACHED = {}


def _build_nc():
    nc = bacc.Bacc("TRN2", target_bir_lowering=False, debug=False)

    zc8_d = nc.dram_tensor("zc8", [ZP, 2, NB, T], FP8, kind="ExternalInput")
    wvha_d = nc.dram_tensor("wvha", [128, NB, N], BF16, kind="ExternalInput")
    wvhb_d = nc.dram_tensor("wvhb", [KA - 128, NB, N], BF16, kind="ExternalInput")
    arma_d = nc.dram_tensor("arma", [128, NB, 3 * T], BF16, kind="ExternalInput")
    armb_d = nc.dram_tensor("armb", [KA - 128, NB, 3 * T], BF16,
                            kind="ExternalInput")
    wt8_d = nc.dram_tensor("wt8", [26, 2, NB, N], FP8, kind="ExternalInput")
    ar8_d = nc.dram_tensor("ar8", [26, 2, NB, 3, 3, T], FP8, kind="ExternalInput")
    d8_d = nc.dram_tensor("d8", [NB, ZP, 2, 3, N], FP8, kind="ExternalInput")
    out_d = nc.dram_tensor("out", [NB, NCH, 128, 3 * T], BF16,
                           kind="ExternalOutput")

    with tile.TileContext(nc) as tc:
        with (
            tc.tile_pool(name="const", bufs=1) as p_const,
            tc.tile_pool(name="glob", bufs=1) as p_glob,
            tc.tile_pool(name="samp", bufs=2) as p_samp,
            tc.tile_pool(name="work", bufs=4) as p_work,
            tc.tile_pool(name="psm", bufs=2, space="PSUM") as ps_m,
            tc.tile_pool(name="psts", bufs=1, space="PSUM") as ps_ts,
        ):

            # ---- gating DMAs for chunk 0 (dispatch slots are ~650ns each
            # and DMA-completion semaphores add ~900ns, so order matters):
            # d8 piece 1 first (it gates the dv->dvs->pm8 prologue chain),
            # then zc8[0], wt8[0], ar8[0].
            d8_t = [p_glob.tile([ZP, 2, 3, N], FP8, tag=f"d8_{i}",
                                name=f"d8_{i}") for i in range(NB)]
            nc.sync.dma_start(d8_t[0][:, :, :, 0:D8SPLIT],
                              d8_d[0, :, :, :, 0:D8SPLIT])
            zc8 = p_glob.tile([ZP, 2, NB, T], FP8)
            nc.sync.dma_start(zc8[:, :, 0], zc8_d[:, :, 0])
            wt8 = p_glob.tile([26, 2, NB, N], FP8)
            nc.sync.dma_start(wt8[:, :, 0], wt8_d[:, :, 0])
            ar8 = p_glob.tile([26, 2, NB, 3, 3, T], FP8)
            nc.sync.dma_start(ar8[:, :, 0], ar8_d[:, :, 0])
            nc.sync.dma_start(d8_t[0][:, :, :, D8SPLIT:2 * D8SPLIT],
                              d8_d[0, :, :, :, D8SPLIT:2 * D8SPLIT])
            nc.sync.dma_start(d8_t[0][:, :, :, 2 * D8SPLIT:4 * D8SPLIT],
                              d8_d[0, :, :, :, 2 * D8SPLIT:4 * D8SPLIT])

            # ---- main-path inputs for sample 0 (first halves first: mains(0)
            # only needs n-columns of the current chunk).
            wvha_t = [p_glob.tile([128, N], BF16, tag=f"wvha_{i}",
                                  name=f"wvha_{i}") for i in range(NB)]
            wvhb_t = [p_glob.tile([KA - 128, N], BF16, tag=f"wvhb_{i}",
                                  name=f"wvhb_{i}") for i in range(NB)]
            arma_t = [p_glob.tile([128, 3 * T], BF16, tag=f"arma_{i}",
                                  name=f"arma_{i}") for i in range(NB)]
            armb_t = [p_glob.tile([KA - 128, 3 * T], BF16, tag=f"armb_{i}",
                                  name=f"armb_{i}") for i in range(NB)]
            nc.sync.dma_start(arma_t[0][:], arma_d[:, 0])
            nc.sync.dma_start(armb_t[0][:], armb_d[:, 0])
            NH = N // 2
            nc.sync.dma_start(wvha_t[0][:, 0:NH], wvha_d[:, 0, 0:NH])
            nc.sync.dma_start(wvhb_t[0][:, 0:NH], wvhb_d[:, 0, 0:NH])
            nc.sync.dma_start(d8_t[0][:, :, :, 4 * D8SPLIT:N],
                              d8_d[0, :, :, :, 4 * D8SPLIT:N])
            nc.sync.dma_start(wvha_t[0][:, NH:N], wvha_d[:, 0, NH:N])
            nc.sync.dma_start(wvhb_t[0][:, NH:N], wvhb_d[:, 0, NH:N])
            # remaining small correction inputs for samples 1..3
            nc.sync.dma_start(zc8[:, :, 1:NB], zc8_d[:, :, 1:NB])
            nc.sync.dma_start(wt8[:, :, 1:NB], wt8_d[:, :, 1:NB])
            nc.sync.dma_start(ar8[:, :, 1:NB], ar8_d[:, :, 1:NB])

            def main_dmas(nb):
                nc.sync.dma_start(wvha_t[nb][:], wvha_d[:, nb])
                nc.sync.dma_start(arma_t[nb][:], arma_d[:, nb])
                nc.sync.dma_start(wvhb_t[nb][:], wvhb_d[:, nb])
                nc.sync.dma_start(armb_t[nb][:], armb_d[:, nb])

            identb = p_const.tile([128, 128], BF16)
            nc.gpsimd.memset(identb[:], 0.0)
            nc.gpsimd.affine_select(
                out=identb[:], in_=identb[:],
                compare_op=mybir.AluOpType.not_equal,
                fill=1.0 / DS, base=0, pattern=[[-1, 128]],
                channel_multiplier=1)
            ident8 = p_const.tile([128, 128], FP8)
            nc.scalar.copy(ident8[:], identb[:])
            ident8dr = p_const.tile([128, 2, 128], FP8)
            nc.scalar.copy(ident8dr[:, 0, :], ident8[:])
            nc.scalar.copy(ident8dr[:, 1, :], ident8[:])

            # ---- PE pre-warm: dummy DR matmuls (junk into the first pM pool
            # buffer, overwritten by chunk 0's start=True mains) so the 3us
            # pstate ramp completes while the gating DMAs stream in.
            pm_warm = ps_m.tile([128, 3 * T], F32, tag="M")
            for _ in range(10):
                nc.tensor.matmul(pm_warm[:, 0:128], ident8dr[:], ident8dr[:],
                                 start=True, stop=True,
                                 perf_mode=mybir.MatmulPerfMode.DoubleRow)

            # cb [128, 3(j), 512] f32 x2: cols 0:384 = ts8(k) slab (i,t) for
            # k%2 == b; cols 384:512 (the otherwise-wasted bank pads) hold
            # dv(k+1) -- the CROSSED buffer.  Each tile thus gets one write
            # burst (ts8(k+1) then dv(k+2), back-to-back on PE at the top of
            # window k) followed by its readers (pm8(k+1) next window,
            # dvs(k+2) this window), so with the tile-granular dependency
            # tracker no write ever waits behind a long read: the pm8 train
            # runs back-to-back at 1325ns.
            cb_t = [ps_ts.tile([128, 3, 512], F32, name=f"cb{i}")
                    for i in range(2)]
            dvs_t = [None] * 4  # ring of 4, indexed g % 4

            def ts8_chunk(g):
                nb, nch = divmod(g, NCH)
                nsl = slice(nch * 128, nch * 128 + 128)
                cb = cb_t[g % 2]
                with tc.high_priority():
                    for j in range(3):
                        nc.tensor.matmul(
                            cb[:, j, 0:384], wt8[:, :, nb, nsl],
                            ar8[:, :, nb, j].rearrange("k u i t -> k u (i t)"),
                            start=True, stop=True,
                            perf_mode=mybir.MatmulPerfMode.DoubleRow,
                        )

            def dv_chunk(g):
                nb, nch = divmod(g, NCH)
                nsl = slice(nch * 128, nch * 128 + 128)
                cb = cb_t[(g + 1) % 2]      # crossed: pads of the other buffer
                with tc.high_priority():
                    for j in range(3):
                        nc.tensor.matmul(
                            cb[:, j, 384:512], d8_t[nb][:, :, j, nsl],
                            zc8[:, :, nb], start=True, stop=True,
                            perf_mode=mybir.MatmulPerfMode.DoubleRow,
                        )
                dvs = p_work.tile([128, 3, T], BF16, tag="dvs")
                with tc.high_priority():
                    nc.scalar.copy(dvs[:], cb[:, :, 384:512])
                dvs_t[g % 4] = dvs

            outacc_t = [None] * NB
            outacc_t[0] = p_samp.tile([128, NCH, 3 * T], BF16,
                                      tag="outacc", name="outacc")

            # software-pipeline prologue (depth 3).  ts8(0) first: it only
            # needs wt8/ar8, which land before d8.
            ts8_chunk(0)
            dv_chunk(0)
            ts8_chunk(1)
            dv_chunk(1)
            dv_chunk(2)

            NTOT = NB * NCH
            OUTB = 4  # chunks per output DMA burst
            pend = []  # staged chunks: pm8 -> (main+reduce) -> evac
            for gi in range(NTOT + 3):
                if gi < NTOT:
                    nb, nch = divmod(gi, NCH)
                    if nch == 2 and nb + 1 < NB:
                        nc.sync.dma_start(d8_t[nb + 1][:], d8_d[nb + 1])
                    if nch == 10 and nb + 1 < NB:
                        main_dmas(nb + 1)
                        outacc_t[nb + 1] = p_samp.tile(
                            [128, NCH, 3 * T], BF16, tag="outacc",
                            name="outacc")

                if pend and pend[-1][4] == 0:
                    # main + j-reduce of chunk gi-1 (PE, top of window gi)
                    _, ppm8, pnb, pnch, _st = pend[-1]
                    pM = ps_m.tile([128, 3 * T], F32, tag="M")
                    pnsl = slice(pnch * 128, pnch * 128 + 128)
                    nc.tensor.matmul(pM[:], wvha_t[pnb][:, pnsl],
                                     arma_t[pnb][:], start=True, stop=False)
                    nc.tensor.matmul(pM[:], wvhb_t[pnb][:, pnsl],
                                     armb_t[pnb][:], start=False, stop=False)
                    nc.tensor.matmul(
                        pM[:], ident8dr[:],
                        ppm8[:, 0:2].rearrange("n j i t -> n j (i t)"),
                        start=False, stop=False,
                        perf_mode=mybir.MatmulPerfMode.DoubleRow,
                        skip_group_check=True,
                    )
                    nc.tensor.matmul(
                        pM[:], ident8[:],
                        ppm8[:, 2].rearrange("n i t -> n (i t)"),
                        start=False, stop=True, skip_group_check=True,
                    )
                    pend[-1] = [gi - 1, pM, pnb, pnch, 1]

                if gi < NTOT:
                    # pm8 [128, 3(j), 3(i), T] fp8 = ts8 * dvs (DVE); both
                    # inputs were produced a window ago -- never stalls.
                    pm8 = p_work.tile([128, 3, 3, T], FP8, tag="pm8")
                    nc.vector.tensor_tensor(
                        pm8[:],
                        cb_t[gi % 2][:, :, 0:384].rearrange(
                            "n j (i t) -> n j i t", i=3),
                        dvs_t[gi % 4][:].unsqueeze(2).broadcast_to(
                            (128, 3, 3, T)),
                        mybir.AluOpType.mult,
                    )
                    pend.append([gi, pm8, nb, nch, 0])
                    if gi + 2 < NTOT:
                        ts8_chunk(gi + 2)
                    if gi + 3 < NTOT:
                        dv_chunk(gi + 3)

                if pend and pend[0][4] == 1 and (gi - pend[0][0] >= 2
                                                 or gi >= NTOT):
                    # evac of chunk gi-2 (ACT: two behind, so the evac's
                    # reduce-wait never delays the next dvs in ACT's queue)
                    _, pM2, pnb2, pnch2, _st = pend.pop(0)
                    nc.scalar.copy(outacc_t[pnb2][:, pnch2, :], pM2[:])
                    ob = (1 if pnb2 == NB - 1 and pnch2 >= NCH - 2 else
          2 if pnb2 == NB - 1 and pnch2 >= NCH - 4 else OUTB)
                    if pnch2 % ob == ob - 1:
                        c0 = pnch2 - (ob - 1)
                        nc.sync.dma_start(
                            out_d[pnb2, c0:pnch2 + 1].rearrange(
                                "c p f -> p c f"),
                            outacc_t[pnb2][:, c0:pnch2 + 1],
                        )


    nc.compile()
    return nc


def _rodrigues_feat(pose):
    # pose: [NB, T, JB, 3] float32 -> (R - I) flattened [NB, T, PF]
    aa = pose.astype(np.float32)
    angle = np.sqrt((aa * aa).sum(-1, keepdims=True))            # [NB,T,JB,1]
    axis = aa / np.maximum(angle, 1e-8)
    x, y, z = axis[..., 0], axis[..., 1], axis[..., 2]
    s = np.sin(angle[..., 0])[..., None, None]
    c = np.cos(angle[..., 0])[..., None, None]
    zero = np.zeros_like(x)
    K = np.stack([
        np.stack([zero, -z, y], axis=-1),
        np.stack([z, zero, -x], axis=-1),
        np.stack([-y, x, zero], axis=-1),
    ], axis=-2)
    outer = axis[..., :, None] * axis[..., None, :]
    I = np.eye(3, dtype=np.float32)
    R = c * I + s * K + (1.0 - c) * outer
    return (R - I).reshape(aa.shape[0], aa.shape[1], PF)


def _prep_core(c, pose_body, trans, betas, A, v_template, shapedirs, posedirs,
               lbs_weights):
    bs = slice(NB * c, NB * (c + 1))

    # zc = [pose_feature | betas] per (sample, t), packed into fp8 dual-pairs
    # (z, z+103); the phantom row z=205 is the zero at (102, u=1).
    pf = _rodrigues_feat(pose_body[bs].reshape(NB, T, JB, 3))    # [NB,T,PF]
    zc = np.concatenate([pf, betas[bs]], axis=2)                 # [NB,T,Z]
    zcT = np.ascontiguousarray(zc.transpose(2, 0, 1))            # [Z,NB,T]
    zc8 = np.zeros((ZP, 2, NB, T), np.float32)
    zc8[:, 0] = zcT[0:ZP]
    zc8[0:Z - ZP, 1] = zcT[ZP:Z]
    zc8 = zc8.astype(NPF8)

    wt = np.concatenate(
        [lbs_weights[bs].transpose(0, 2, 1),
         np.ones((NB, 1, N), np.float32)], axis=1)             # [NB, 53, N]
    vth = np.concatenate(
        [v_template[bs], np.ones((NB, N, 1), np.float32)], axis=2)  # [NB,N,4]
    wvh = (vth.transpose(0, 2, 1)[:, :, None, :] * wt[:, None, :, :]
           ).reshape(NB, KA, N)                                # [NB,(j,k),N]
    wvha = np.ascontiguousarray(wvh[:, 0:128].transpose(1, 0, 2)).astype(NPBF16)
    wvhb = np.ascontiguousarray(wvh[:, 128:KA].transpose(1, 0, 2)).astype(NPBF16)

    arm = np.zeros((NB, 4, J + 1, 3, T), np.float32)
    arm[:, :, :J] = A[bs, :, :, 0:3, :].transpose(0, 4, 2, 3, 1)  # [nb,j,k,i,t]
    arm[:, 3, J] = trans[bs].transpose(0, 2, 1)                   # [nb,i,t]
    arm = arm.reshape(NB, KA, 3 * T)
    arma = np.ascontiguousarray(arm[:, 0:128].transpose(1, 0, 2)).astype(NPBF16)
    armb = np.ascontiguousarray(arm[:, 128:KA].transpose(1, 0, 2)).astype(NPBF16)

    wt8 = np.empty((26, 2, NB, N), np.float32)
    wt8[:, 0] = wt[:, 0:26].transpose(1, 0, 2)
    wt8[:, 1] = wt[:, 26:52].transpose(1, 0, 2)
    wt8 = wt8.astype(NPF8)

    ar8f = A[bs, :, :, 0:3, 0:3].transpose(0, 2, 4, 3, 1)      # [nb,k,j,i,t]
    ar8 = np.empty((26, 2, NB, 3, 3, T), np.float32)
    ar8[:, 0] = ar8f[:, 0:26].transpose(1, 0, 2, 3, 4)
    ar8[:, 1] = ar8f[:, 26:52].transpose(1, 0, 2, 3, 4)
    ar8 = ar8.astype(NPF8)

    D = np.concatenate([
        posedirs[bs].reshape(NB, PF, N, 3),
        shapedirs[bs].transpose(0, 3, 1, 2),                   # [NB, L, N, 3]
    ], axis=1)                                                 # [NB, Z, N, 3]
    Dt = D.transpose(0, 1, 3, 2) * DS                          # [NB, Z, 3, N]
    d8 = np.zeros((NB, ZP, 2, 3, N), np.float32)
    d8[:, :, 0] = Dt[:, 0:ZP]
    d8[:, 0:Z - ZP, 1] = Dt[:, ZP:Z]
    d8 = d8.astype(NPF8)

    return {
        "zc8": zc8, "wvha": wvha, "wvhb": wvhb,
        "arma": arma, "armb": armb, "wt8": wt8, "ar8": ar8, "d8": d8,
    }


def kernel(pose_body, trans, betas, A, v_template, shapedirs, posedirs,
           lbs_weights):
    if "nc" not in _CACHED:
        _CACHED["nc"] = _build_nc()
    nc = _CACHED["nc"]

    args = (pose_body, trans, betas, A, v_template, shapedirs, posedirs,
            lbs_weights)
    args = tuple(np.asarray(a, dtype=np.float32) for a in args)
    in_maps = [_prep_core(c, *args) for c in range(NCORES)]

    res = bass_utils.run_bass_kernel_spmd(nc, in_maps,
                                          core_ids=list(range(NCORES)))

    # out [NB, NCH, 128, 3*T] per core -> (B, T, N, 3)
    full = np.stack(
        [res.results[c]["out"].astype(np.float32) for c in range(NCORES)]
    )
    full = full.reshape(B, NCH, 128, 3, T).transpose(0, 4, 1, 2, 3)
    return np.ascontiguousarray(full.reshape(B, T, N, 3).astype(np.float32))


# revision 15
# speedup vs baseline: 1.0131x; 1.0131x over previous
"""Trainium2 Bass kernel for MinimalLBS (B=32, T=128, N=2048, J=52, Jb=21, L=16).

Data-parallel over B across 8 NeuronCores (4 samples per core).

Device math per sample (per 128-vertex chunk, t free):
  MAIN (exact, bf16 on PE): fold v_template+homogeneous into the stationary:
     M[n,(i,t)] = sum_{(j,k)} wvh[(j,k),n] * arm[(j,k),(i,t)]
     where wvh[(j,k),n] = wt[k,n]*vth[n,j]  (host-prepped, K=212),
     arm[(j,k),(i,t)] = A[t,k,i,j] (+ translation row at (3,J)).
  CORRECTION (fp8 DoubleRow): dv[n,j,t] (pose+shape offsets, K=206 dual-tile
     over 103 partitions) and ts8[n,(j,i,t)] (K=52 dual-tile) on PE; pm8 =
     ts8*dv on DVE (fp8 out, x32 scale in D8); j-reduction via fp8 identity
     matmuls (ident/32) accumulating straight into the M PSUM bank.

Steady state is DVE-bound: one 1325ns pm8 tensor_tensor per chunk,
back-to-back.  To keep that train unstalled, dv/dvs for chunk g+1 are
computed one window EARLY (software pipeline depth 2 on the correction
inputs), so pm8(g) only ever waits on ts8(g), which lands ~400ns into the
window.  Rodrigues pose features + betas are folded on the HOST into zc8;
PE is pre-warmed with dummy matmuls; gating DMAs are split so chunk 0's
inputs (zc8[0], first 6 n-chunks of d8[0], wt8[0], ar8[0]) land first.
"""

import sys

sys.path.insert(0, "/opt/trn_rl_repo")

import ml_dtypes
import numpy as np

import concourse.bacc as bacc
import concourse.mybir as mybir
import concourse.tile as tile
from concourse import bass_utils, masks

F32 = mybir.dt.float32
BF16 = mybir.dt.bfloat16
FP8 = mybir.dt.float8e4
NPBF16 = ml_dtypes.bfloat16
NPF8 = ml_dtypes.float8_e4m3

B, T, N, JB, J, L = 32, 128, 2048, 21, 52, 16
NCORES = 8
NB = B // NCORES          # samples per core
PF = JB * 9               # 189 pose-feature dims
Z = PF + L                # 205 combined correction coeffs
ZP = 103                  # dual-pair partitions for the z contraction (206>=205)
KA = (J + 1) * 4          # 212 (j,k) rows for the main matmul
NCH = N // 128            # n-chunks per sample
DS = 32.0                 # fp8 scale baked into D8, undone by ident/DS
D8SPLIT = 4 * 128         # first d8[0] DMA piece covers 4 n-chunks

_CACHED = {}


def _build_nc():
    nc = bacc.Bacc("TRN2", target_bir_lowering=False, debug=False)

    zc8_d = nc.dram_tensor("zc8", [ZP, 2, NB, T], FP8, kind="ExternalInput")
    wvha_d = nc.dram_tensor("wvha", [128, NB, N], BF16, kind="ExternalInput")
    wvhb_d = nc.dram_tensor("wvhb", [KA - 128, NB, N], BF16, kind="ExternalInput")
    arma_d = nc.dram_tensor("arma", [128, NB, 3 * T], BF16, kind="ExternalInput")
    armb_d = nc.dram_tensor("armb", [KA - 128, NB, 3 * T], BF16,
                            kind="ExternalInput")
    wt8_d = nc.dram_tensor("wt8", [26, 2, NB, N], FP8, kind="ExternalInput")
    ar8_d = nc.dram_tensor("ar8", [26, 2, NB, 3, 3, T], FP8, kind="ExternalInput")
    d8_d = nc.dram_tensor("d8", [NB, ZP, 2, 3, N], FP8, kind="ExternalInput")
    out_d = nc.dram_tensor("out", [NB, NCH, 128, 3 * T], BF16,
                           kind="ExternalOutput")

    with tile.TileContext(nc) as tc:
        with (
            tc.tile_pool(name="const", bufs=1) as p_const,
            tc.tile_pool(name="glob", bufs=1) as p_glob,
            tc.tile_pool(name="samp", bufs=2) as p_samp,
            tc.tile_pool(name="work", bufs=4) as p_work,
            tc.tile_pool(name="psm", bufs=2, space="PSUM") as ps_m,
            tc.tile_pool(name="psts", bufs=1, space="PSUM") as ps_ts,
        ):

            # ---- gating DMAs for chunk 0 (dispatch slots are ~650ns each
            # and DMA-completion semaphores add ~900ns, so order matters):
            # d8 piece 1 first (it gates the dv->dvs->pm8 prologue chain),
            # then zc8[0], wt8[0], ar8[0].
            d8_t = [p_glob.tile([ZP, 2, 3, N], FP8, tag=f"d8_{i}",
                                name=f"d8_{i}") for i in range(NB)]
            nc.sync.dma_start(d8_t[0][:, :, :, 0:D8SPLIT],
                              d8_d[0, :, :, :, 0:D8SPLIT])
            zc8 = p_glob.tile([ZP, 2, NB, T], FP8)
            nc.sync.dma_start(zc8[:, :, 0], zc8_d[:, :, 0])
            wt8 = p_glob.tile([26, 2, NB, N], FP8)
            nc.sync.dma_start(wt8[:, :, 0], wt8_d[:, :, 0])
            ar8 = p_glob.tile([26, 2, NB, 3, 3, T], FP8)
            nc.sync.dma_start(ar8[:, :, 0], ar8_d[:, :, 0])
            nc.sync.dma_start(d8_t[0][:, :, :, D8SPLIT:2 * D8SPLIT],
                              d8_d[0, :, :, :, D8SPLIT:2 * D8SPLIT])

            # ---- main-path inputs for sample 0 (first halves first: mains(0)
            # only needs n-columns of the current chunk).
            wvha_t = [p_glob.tile([128, N], BF16, tag=f"wvha_{i}",
                                  name=f"wvha_{i}") for i in range(NB)]
            wvhb_t = [p_glob.tile([KA - 128, N], BF16, tag=f"wvhb_{i}",
                                  name=f"wvhb_{i}") for i in range(NB)]
            arma_t = [p_glob.tile([128, 3 * T], BF16, tag=f"arma_{i}",
                                  name=f"arma_{i}") for i in range(NB)]
            armb_t = [p_glob.tile([KA - 128, 3 * T], BF16, tag=f"armb_{i}",
                                  name=f"armb_{i}") for i in range(NB)]
            nc.sync.dma_start(arma_t[0][:], arma_d[:, 0])
            nc.sync.dma_start(armb_t[0][:], armb_d[:, 0])
            NH = N // 2
            nc.sync.dma_start(wvha_t[0][:, 0:NH], wvha_d[:, 0, 0:NH])
            nc.sync.dma_start(wvhb_t[0][:, 0:NH], wvhb_d[:, 0, 0:NH])
            nc.sync.dma_start(d8_t[0][:, :, :, 2 * D8SPLIT:N],
                              d8_d[0, :, :, :, 2 * D8SPLIT:N])
            nc.sync.dma_start(wvha_t[0][:, NH:N], wvha_d[:, 0, NH:N])
            nc.sync.dma_start(wvhb_t[0][:, NH:N], wvhb_d[:, 0, NH:N])
            # remaining small correction inputs for samples 1..3
            nc.sync.dma_start(zc8[:, :, 1:NB], zc8_d[:, :, 1:NB])
            nc.sync.dma_start(wt8[:, :, 1:NB], wt8_d[:, :, 1:NB])
            nc.sync.dma_start(ar8[:, :, 1:NB], ar8_d[:, :, 1:NB])

            def main_dmas(nb):
                nc.sync.dma_start(wvha_t[nb][:], wvha_d[:, nb])
                nc.sync.dma_start(arma_t[nb][:], arma_d[:, nb])
                nc.sync.dma_start(wvhb_t[nb][:], wvhb_d[:, nb])
                nc.sync.dma_start(armb_t[nb][:], armb_d[:, nb])

            identb = p_const.tile([128, 128], BF16)
            nc.gpsimd.memset(identb[:], 0.0)
            nc.gpsimd.affine_select(
                out=identb[:], in_=identb[:],
                compare_op=mybir.AluOpType.not_equal,
                fill=1.0 / DS, base=0, pattern=[[-1, 128]],
                channel_multiplier=1)
            ident8 = p_const.tile([128, 128], FP8)
            nc.scalar.copy(ident8[:], identb[:])
            ident8dr = p_const.tile([128, 2, 128], FP8)
            nc.scalar.copy(ident8dr[:, 0, :], ident8[:])
            nc.scalar.copy(ident8dr[:, 1, :], ident8[:])

            # ---- PE pre-warm: dummy DR matmuls (junk into the first pM pool
            # buffer, overwritten by chunk 0's start=True mains) so the 3us
            # pstate ramp completes while the gating DMAs stream in.
            pm_warm = ps_m.tile([128, 3 * T], F32, tag="M")
            for _ in range(10):
                nc.tensor.matmul(pm_warm[:, 0:128], ident8dr[:], ident8dr[:],
                                 start=True, stop=True,
                                 perf_mode=mybir.MatmulPerfMode.DoubleRow)

            # cb [128, 3(j), 512] f32 x2: cols 0:384 = ts8(k) slab (i,t) for
            # k%2 == b; cols 384:512 (the otherwise-wasted bank pads) hold
            # dv(k+1) -- the CROSSED buffer.  Each tile thus gets one write
            # burst (ts8(k+1) then dv(k+2), back-to-back on PE at the top of
            # window k) followed by its readers (pm8(k+1) next window,
            # dvs(k+2) this window), so with the tile-granular dependency
            # tracker no write ever waits behind a long read: the pm8 train
            # runs back-to-back at 1325ns.
            cb_t = [ps_ts.tile([128, 3, 512], F32, name=f"cb{i}")
                    for i in range(2)]
            dvs_t = [None] * 4  # ring of 4, indexed g % 4

            def ts8_chunk(g):
                nb, nch = divmod(g, NCH)
                nsl = slice(nch * 128, nch * 128 + 128)
                cb = cb_t[g % 2]
                with tc.high_priority():
                    for j in range(3):
                        nc.tensor.matmul(
                            cb[:, j, 0:384], wt8[:, :, nb, nsl],
                            ar8[:, :, nb, j].rearrange("k u i t -> k u (i t)"),
                            start=True, stop=True,
                            perf_mode=mybir.MatmulPerfMode.DoubleRow,
                        )

            def dv_chunk(g):
                nb, nch = divmod(g, NCH)
                nsl = slice(nch * 128, nch * 128 + 128)
                cb = cb_t[(g + 1) % 2]      # crossed: pads of the other buffer
                with tc.high_priority():
                    for j in range(3):
                        nc.tensor.matmul(
                            cb[:, j, 384:512], d8_t[nb][:, :, j, nsl],
                            zc8[:, :, nb], start=True, stop=True,
                            perf_mode=mybir.MatmulPerfMode.DoubleRow,
                        )
                dvs = p_work.tile([128, 3, T], BF16, tag="dvs")
                with tc.high_priority():
                    nc.scalar.copy(dvs[:], cb[:, :, 384:512])
                dvs_t[g % 4] = dvs

            outacc_t = [None] * NB
            outacc_t[0] = p_samp.tile([128, NCH, 3 * T], BF16,
                                      tag="outacc", name="outacc")

            # software-pipeline prologue (depth 3).  dv/dvs first: they are
            # all gated by the first d8 piece and chain through ACT, which is
            # the startup critical path; the ts8s slot in behind them.
            dv_chunk(0)
            dv_chunk(1)
            ts8_chunk(0)
            ts8_chunk(1)
            dv_chunk(2)

            NTOT = NB * NCH
            OUTB = 4  # chunks per output DMA burst
            pend = []  # staged chunks: pm8 -> (main+reduce) -> evac
            for gi in range(NTOT + 3):
                if gi < NTOT:
                    nb, nch = divmod(gi, NCH)
                    if nch == 2 and nb + 1 < NB:
                        nc.sync.dma_start(d8_t[nb + 1][:], d8_d[nb + 1])
                    if nch == 10 and nb + 1 < NB:
                        main_dmas(nb + 1)
                        outacc_t[nb + 1] = p_samp.tile(
                            [128, NCH, 3 * T], BF16, tag="outacc",
                            name="outacc")

                if pend and pend[-1][4] == 0:
                    # main + j-reduce of chunk gi-1 (PE, top of window gi)
                    _, ppm8, pnb, pnch, _st = pend[-1]
                    pM = ps_m.tile([128, 3 * T], F32, tag="M")
                    pnsl = slice(pnch * 128, pnch * 128 + 128)
                    nc.tensor.matmul(pM[:], wvha_t[pnb][:, pnsl],
                                     arma_t[pnb][:], start=True, stop=False)
                    nc.tensor.matmul(pM[:], wvhb_t[pnb][:, pnsl],
                                     armb_t[pnb][:], start=False, stop=False)
                    nc.tensor.matmul(
                        pM[:], ident8dr[:],
                        ppm8[:, 0:2].rearrange("n j i t -> n j (i t)"),
                        start=False, stop=False,
                        perf_mode=mybir.MatmulPerfMode.DoubleRow,
                        skip_group_check=True,
                    )
                    nc.tensor.matmul(
                        pM[:], ident8[:],
                        ppm8[:, 2].rearrange("n i t -> n (i t)"),
                        start=False, stop=True, skip_group_check=True,
                    )
                    pend[-1] = [gi - 1, pM, pnb, pnch, 1]

                if gi < NTOT:
                    # pm8 [128, 3(j), 3(i), T] fp8 = ts8 * dvs (DVE); both
                    # inputs were produced a window ago -- never stalls.
                    pm8 = p_work.tile([128, 3, 3, T], FP8, tag="pm8")
                    nc.vector.tensor_tensor(
                        pm8[:],
                        cb_t[gi % 2][:, :, 0:384].rearrange(
                            "n j (i t) -> n j i t", i=3),
                        dvs_t[gi % 4][:].unsqueeze(2).broadcast_to(
                            (128, 3, 3, T)),
                        mybir.AluOpType.mult,
                    )
                    pend.append([gi, pm8, nb, nch, 0])
                    if gi + 2 < NTOT:
                        ts8_chunk(gi + 2)
                    if gi + 3 < NTOT:
                        dv_chunk(gi + 3)

                if pend and pend[0][4] == 1 and (gi - pend[0][0] >= 2
                                                 or gi >= NTOT):
                    # evac of chunk gi-2 (ACT: two behind, so the evac's
                    # reduce-wait never delays the next dvs in ACT's queue)
                    _, pM2, pnb2, pnch2, _st = pend.pop(0)
                    nc.scalar.copy(outacc_t[pnb2][:, pnch2, :], pM2[:])
                    ob = (1 if pnb2 == NB - 1 and pnch2 >= NCH - 2 else
          2 if pnb2 == NB - 1 and pnch2 >= NCH - 4 else OUTB)
                    if pnch2 % ob == ob - 1:
                        c0 = pnch2 - (ob - 1)
                        nc.sync.dma_start(
                            out_d[pnb2, c0:pnch2 + 1].rearrange(
                                "c p f -> p c f"),
                            outacc_t[pnb2][:, c0:pnch2 + 1],
                        )


    nc.compile()
    return nc


def _rodrigues_feat(pose):
    # pose: [NB, T, JB, 3] float32 -> (R - I) flattened [NB, T, PF]
    aa = pose.astype(np.float32)
    angle = np.sqrt((aa * aa).sum(-1, keepdims=True))            # [NB,T,JB,1]
    axis = aa / np.maximum(angle, 1e-8)
    x, y, z = axis[..., 0], axis[..., 1], axis[..., 2]
    s = np.sin(angle[..., 0])[..., None, None]
    c = np.cos(angle[..., 0])[..., None, None]
    zero = np.zeros_like(x)
    K = np.stack([
        np.stack([zero, -z, y], axis=-1),
        np.stack([z, zero, -x], axis=-1),
        np.stack([-y, x, zero], axis=-1),
    ], axis=-2)
    outer = axis[..., :, None] * axis[..., None, :]
    I = np.eye(3, dtype=np.float32)
    R = c * I + s * K + (1.0 - c) * outer
    return (R - I).reshape(aa.shape[0], aa.shape[1], PF)


def _prep_core(c, pose_body, trans, betas, A, v_template, shapedirs, posedirs,
               lbs_weights):
    bs = slice(NB * c, NB * (c + 1))

    # zc = [pose_feature | betas] per (sample, t), packed into fp8 dual-pairs
    # (z, z+103); the phantom row z=205 is the zero at (102, u=1).
    pf = _rodrigues_feat(pose_body[bs].reshape(NB, T, JB, 3))    # [NB,T,PF]
    zc = np.concatenate([pf, betas[bs]], axis=2)                 # [NB,T,Z]
    zcT = np.ascontiguousarray(zc.transpose(2, 0, 1))            # [Z,NB,T]
    zc8 = np.zeros((ZP, 2, NB, T), np.float32)
    zc8[:, 0] = zcT[0:ZP]
    zc8[0:Z - ZP, 1] = zcT[ZP:Z]
    zc8 = zc8.astype(NPF8)

    wt = np.concatenate(
        [lbs_weights[bs].transpose(0, 2, 1),
         np.ones((NB, 1, N), np.float32)], axis=1)             # [NB, 53, N]
    vth = np.concatenate(
        [v_template[bs], np.ones((NB, N, 1), np.float32)], axis=2)  # [NB,N,4]
    wvh = (vth.transpose(0, 2, 1)[:, :, None, :] * wt[:, None, :, :]
           ).reshape(NB, KA, N)                                # [NB,(j,k),N]
    wvha = np.ascontiguousarray(wvh[:, 0:128].transpose(1, 0, 2)).astype(NPBF16)
    wvhb = np.ascontiguousarray(wvh[:, 128:KA].transpose(1, 0, 2)).astype(NPBF16)

    arm = np.zeros((NB, 4, J + 1, 3, T), np.float32)
    arm[:, :, :J] = A[bs, :, :, 0:3, :].transpose(0, 4, 2, 3, 1)  # [nb,j,k,i,t]
    arm[:, 3, J] = trans[bs].transpose(0, 2, 1)                   # [nb,i,t]
    arm = arm.reshape(NB, KA, 3 * T)
    arma = np.ascontiguousarray(arm[:, 0:128].transpose(1, 0, 2)).astype(NPBF16)
    armb = np.ascontiguousarray(arm[:, 128:KA].transpose(1, 0, 2)).astype(NPBF16)

    wt8 = np.empty((26, 2, NB, N), np.float32)
    wt8[:, 0] = wt[:, 0:26].transpose(1, 0, 2)
    wt8[:, 1] = wt[:, 26:52].transpose(1, 0, 2)
    wt8 = wt8.astype(NPF8)

    ar8f = A[bs, :, :, 0:3, 0:3].transpose(0, 2, 4, 3, 1)      # [nb,k,j,i,t]
    ar8 = np.empty((26, 2, NB, 3, 3, T), np.float32)
    ar8[:, 0] = ar8f[:, 0:26].transpose(1, 0, 2, 3, 4)
    ar8[:, 1] = ar8f[:, 26:52].transpose(1, 0, 2, 3, 4)
    ar8 = ar8.astype(NPF8)

    D = np.concatenate([
        posedirs[bs].reshape(NB, PF, N, 3),
        shapedirs[bs].transpose(0, 3, 1, 2),                   # [NB, L, N, 3]
    ], axis=1)                                                 # [NB, Z, N, 3]
    Dt = D.transpose(0, 1, 3, 2) * DS                          # [NB, Z, 3, N]
    d8 = np.zeros((NB, ZP, 2, 3, N), np.float32)
    d8[:, :, 0] = Dt[:, 0:ZP]
    d8[:, 0:Z - ZP, 1] = Dt[:, ZP:Z]
    d8 = d8.astype(NPF8)

    return {
        "zc8": zc8, "wvha": wvha, "wvhb": wvhb,
        "arma": arma, "armb": armb, "wt8": wt8, "ar8": ar8, "d8": d8,
    }


def kernel(pose_body, trans, betas, A, v_template, shapedirs, posedirs,
           lbs_weights):
    if "nc" not in _CACHED:
        _CACHED["nc"] = _build_nc()
    nc = _CACHED["nc"]

    args = (pose_body, trans, betas, A, v_template, shapedirs, posedirs,
            lbs_weights)
    args = tuple(np.asarray(a, dtype=np.float32) for a in args)
    in_maps = [_prep_core(c, *args) for c in range(NCORES)]

    res = bass_utils.run_bass_kernel_spmd(nc, in_maps,
                                          core_ids=list(range(NCORES)))

    # out [NB, NCH, 128, 3*T] per core -> (B, T, N, 3)
    full = np.stack(
        [res.results[c]["out"].astype(np.float32) for c in range(NCORES)]
    )
    full = full.reshape(B, NCH, 128, 3, T).transpose(0, 4, 1, 2, 3)
    return np.ascontiguousarray(full.reshape(B, T, N, 3).astype(np.float32))
